# revision 17
# baseline (speedup 1.0000x reference)
"""nn_MultiHeadAttention kernel for 8 Trainium2 NeuronCores.

Sharding: 8 cores = 4 batches (data parallel) x 2 head-groups of 8 heads
(tensor parallel). Each core computes its batch's QKV projection for its
head group (column-parallel), RoPE, causal attention, and a partial
out-projection (row-parallel). Host sums the two partials per batch and
adds the output bias.

Design (vs the DRAM-spill f32r baseline):
  - All matmul operands bf16 (host pre-converts); fp32 PSUM accumulation.
  - Everything SBUF-resident: x (8MB), q/k rows (8MB, roped in place),
    v (4MB), attn (4MB). No DRAM scratch roundtrips.
  - Weights host-prearranged to [128, ...] layouts so every weight DMA is
    a contiguous slice (4KB/partition lines).
  - Phase order: V-proj (first Q tile interleaved to hide the wv chunk
    reload), Q/K-proj (head-major), per-head attention, out-proj (first
    weight chunk prefetched during attention). RoPE runs in place on the
    q/k tiles via a partition-swap SBUF-SBUF DMA + 3 DVE ops, emitted
    mid-stream so it never blocks the DVE FIFO at a head start.
  - Attention processes two heads' chunks interleaved step-by-step so the
    PE always has independent work while ScalarE runs exp, with score
    matmuls one j-pair ahead of the PV matmuls. Causal structure:
    j-tiles in descending order with diagonal tiles width-narrowed; only
    the 128x128 diagonal block is masked, by accumulating a shared
    triangular table through a 128-row identity matmul.
  - Scores stay transposed S^T[k,q]; exp on ScalarE straight from PSUM
    with the 1/sqrt(D) scale fused. The softmax denominator accumulates
    on the DVE (SBUF bf16 tile, one add per pexp tile) with a single
    ones-column matmul per chunk — keeping PE work per pipeline cycle
    well below the ScalarE exp pace so HAM clock-gate oscillation cannot
    throttle the PE (this alone was worth ~73us/iter).
  - Normalization is decoupled from the PSUM critical path: unnormalized
    P@V evicts immediately (DVE); the 1/Z multiply (DVE reciprocal +
    GPSIMD partition-broadcast) is deferred one chunk.
"""

import sys

if "/opt/trn_rl_repo" not in sys.path:
    sys.path.insert(0, "/opt/trn_rl_repo")

import numpy as np
import ml_dtypes

import concourse.bass as bass
import concourse.bacc as bacc
import concourse.mybir as mybir
import concourse.tile as tile
from concourse.bass_utils import run_bass_kernel_spmd

F32 = mybir.dt.float32
BF16 = mybir.dt.bfloat16
BF_NP = ml_dtypes.bfloat16

B, T, C = 4, 2048, 2048
H = 16            # total heads
HG = 8            # heads per core (group)
D = 128           # head dim
GC = HG * D       # channels per group = 1024
SCALE = 1.0 / float(np.sqrt(D))
MASKVAL = -30000.0
N_CORES = 8

KT = C // 128     # 16 contraction tiles
TT = T // 128     # 16 T tiles
TC = T // 512     # 4 T chunks of 512
ND = 2            # v output chunks of 512


def build_program(iters=1):
    nc = bacc.Bacc("TRN2", target_bir_lowering=False, debug=False)

    xT = nc.dram_tensor("xT", [C, T], BF16, kind="ExternalInput").ap()
    # m-major: [p=128, m(16: q0..q7,k0..k7), k(16), c(128)]
    wqk = nc.dram_tensor("wqk", [128, 16 * KT * 128], BF16,
                         kind="ExternalInput").ap()
    bqk = nc.dram_tensor("bqk", [2 * GC, 1], F32, kind="ExternalInput").ap()
    # nd-major: [p=128, nd(2), k(16), c(512)]
    wv = nc.dram_tensor("wv", [128, ND * KT * 512], BF16,
                        kind="ExternalInput").ap()
    bv = nc.dram_tensor("bv", [1, GC], BF16, kind="ExternalInput").ap()
    # n-major: [p=128, n(4), h(8), c(512)]
    wo = nc.dram_tensor("wo", [128, 4 * HG * 512], BF16,
                        kind="ExternalInput").ap()
    sin2 = nc.dram_tensor("sin2", [128, T], BF16, kind="ExternalInput").ap()
    cos2 = nc.dram_tensor("cos2", [128, T], BF16, kind="ExternalInput").ap()
    # additive upper-triangle mask for the 128x128 diagonal blocks
    tri = nc.dram_tensor("tri", [128, 128], BF16, kind="ExternalInput").ap()
    ident = nc.dram_tensor("ident", [128, 128], BF16,
                           kind="ExternalInput").ap()
    onescol = nc.dram_tensor("onescol", [128, 1], BF16,
                             kind="ExternalInput").ap()
    ones128 = nc.dram_tensor("ones128", [1, 128], BF16,
                             kind="ExternalInput").ap()
    y = nc.dram_tensor("y", [T, C], F32, kind="ExternalOutput").ap()

    with tile.TileContext(nc) as tc:
        with tc.tile_pool(name="consts", bufs=1) as rpool, \
             tc.tile_pool(name="qkp", bufs=1) as qkpool, \
             tc.tile_pool(name="vp", bufs=1) as vpool, \
             tc.tile_pool(name="ropep", bufs=1) as ropepool:
            tri_sb = rpool.tile([128, 128], BF16, tag="tri")
            ident_sb = rpool.tile([128, 128], BF16, tag="ident")
            onescol_sb = rpool.tile([128, 1], BF16, tag="onescol")
            ones128_sb = rpool.tile([1, 128], BF16, tag="ones128")
            bv_sb = rpool.tile([1, GC], BF16, tag="bv")
            sin_sb = rpool.tile([128, T], BF16, tag="sin")
            cos_sb = rpool.tile([128, T], BF16, tag="cos")

            qk_sb = [qkpool.tile([128, T], BF16, tag=f"qk{m}", name=f"qk{m}")
                     for m in range(16)]
            v_sb = [vpool.tile([128, GC], BF16, tag=f"v{t}", name=f"v{t}")
                    for t in range(TT)]

            def emit_rope(h):
                # in-place RoPE on qk_sb[h] (q) and qk_sb[8+h] (k):
                #   roped = raw * cos2 + swapped(raw) * sin2
                for m in (h, 8 + h):
                    sw = ropepool.tile([128, T], BF16, tag="sw", bufs=1,
                                       name=f"sw{m}")
                    nc.sync.dma_start(out=sw[0:64, :], in_=qk_sb[m][64:128, :])
                    nc.sync.dma_start(out=sw[64:128, :], in_=qk_sb[m][0:64, :])
                    tmp = ropepool.tile([128, T], BF16, tag="tmp", bufs=1,
                                        name=f"tmp{m}")
                    nc.vector.tensor_mul(tmp[:], sw[:], sin_sb[:])
                    nc.vector.tensor_mul(sw[:], qk_sb[m][:], cos_sb[:])
                    nc.vector.tensor_add(qk_sb[m][:], tmp[:], sw[:])

            def full_body(iv):
                nc.sync.dma_start(out=tri_sb[:], in_=tri)
                nc.sync.dma_start(out=ident_sb[:], in_=ident)
                nc.sync.dma_start(out=onescol_sb[:], in_=onescol)
                nc.sync.dma_start(out=ones128_sb[:], in_=ones128)
                nc.sync.dma_start(out=bv_sb[:], in_=bv)
                nc.sync.dma_start(out=sin_sb[:], in_=sin2)
                nc.sync.dma_start(out=cos_sb[:], in_=cos2)

                # ---------- Phases V + QK (x resident, weights streamed) ----
                with tc.tile_pool(name="xp", bufs=1) as xpool, \
                     tc.tile_pool(name="wvp", bufs=1) as wvpool, \
                     tc.tile_pool(name="w1p", bufs=1) as w1pool, \
                     tc.tile_pool(name="ps1", bufs=1, space="PSUM") as ps1:
                    xt_sb = []
                    for k in range(KT):
                        t = xpool.tile([128, T], BF16, tag=f"xt{k}",
                                       name=f"xt{k}")
                        nc.sync.dma_start(out=t[:],
                                          in_=xT[k * 128:(k + 1) * 128, :])
                        xt_sb.append(t)

                    def emit_v_chunk(nd):
                        # v[:, nd*512:(nd+1)*512] = x @ Wv chunk + bias
                        wvc = wvpool.tile([128, KT * 512], BF16, tag="wvc",
                                          bufs=1, name=f"wvc{nd}")
                        nc.sync.dma_start(
                            out=wvc[:],
                            in_=wv[:, nd * KT * 512:(nd + 1) * KT * 512])
                        ndsl = slice(nd * 512, (nd + 1) * 512)
                        for t in range(TT):
                            ps = ps1.tile([128, 512], F32, tag="ps1", bufs=4,
                                          name=f"psv{nd}_{t}")
                            for k in range(KT):
                                nc.tensor.matmul(
                                    ps[:],
                                    xt_sb[k][:, t * 128:(t + 1) * 128],
                                    wvc[:, k * 512:(k + 1) * 512],
                                    start=(k == 0), stop=False)
                            nc.tensor.matmul(
                                ps[:], ones128_sb[:], bv_sb[:, ndsl],
                                start=False, stop=True)
                            nc.scalar.copy(v_sb[t][:, ndsl], ps[:])

                    def emit_qk_tile(m):
                        # qk_sb[m] = (x^T @ Wqk col-block m)^T + bias  ([d, T])
                        wrow = w1pool.tile([128, KT * 128], BF16, tag="wrow",
                                           bufs=2, name=f"wrow{m}")
                        nc.sync.dma_start(
                            out=wrow[:], in_=wqk[:, m * 2048:(m + 1) * 2048])
                        bias_t = w1pool.tile([128, 1], F32, tag="bias",
                                             bufs=2, name=f"bias{m}")
                        nc.sync.dma_start(
                            out=bias_t[:], in_=bqk[m * 128:(m + 1) * 128, :])
                        for n in range(TC):
                            ps = ps1.tile([128, 512], F32, tag="ps1", bufs=4,
                                          name=f"psqk{m}_{n}")
                            for k in range(KT):
                                nc.tensor.matmul(
                                    ps[:],
                                    wrow[:, k * 128:(k + 1) * 128],
                                    xt_sb[k][:, n * 512:(n + 1) * 512],
                                    start=(k == 0), stop=(k == KT - 1))
                            nc.vector.tensor_scalar_add(
                                qk_sb[m][:, n * 512:(n + 1) * 512],
                                ps[:], bias_t[:])

                    emit_v_chunk(0)
                    emit_qk_tile(0)       # hides the wvc reload for nd=1
                    emit_v_chunk(1)
                    emit_qk_tile(8)
                    emit_rope(0)
                    for h in range(1, HG):
                        emit_qk_tile(h)
                        emit_qk_tile(8 + h)
                        if h == 1:
                            emit_rope(1)

                # ---------------- Phases 2+3 ----------------
                with tc.tile_pool(name="attnp", bufs=1) as apool, \
                     tc.tile_pool(name="w3p", bufs=1) as w3pool:
                    attn_sb = [
                        apool.tile([128, T], BF16, tag=f"at{h}", name=f"at{h}")
                        for h in range(HG)
                    ]
                    # first out-proj weight chunk prefetches during phase 2
                    woc0 = w3pool.tile([128, HG * 512], BF16, tag="woc",
                                       bufs=2, name="woc0")
                    nc.sync.dma_start(out=woc0[:], in_=wo[:, 0:HG * 512])

                    # ---- Phase 2: attention per head ----
                    with tc.tile_pool(name="pexpp", bufs=1) as epool, \
                         tc.tile_pool(name="normp", bufs=1) as npool, \
                         tc.tile_pool(name="ps2s", bufs=2,
                                      space="PSUM") as ps2, \
                         tc.tile_pool(name="ps2o", bufs=2,
                                      space="PSUM") as po2, \
                         tc.tile_pool(name="ps2z", bufs=2,
                                      space="PSUM") as pz2:
                        def make_chunk(h, n):
                            """Emitter for head h, q-chunk n. step(i) emits
                            scores+exp for j-pair i and PV/Z for pair i-1;
                            tail() emits the last PV/Z and the softmax
                            normalization. Two heads' emitters are
                            interleaved step-by-step so the PE always has
                            independent work while ScalarE runs exp."""
                            qr = qk_sb[h]
                            kr = qk_sb[8 + h]
                            jmax = 4 * (n + 1)
                            ps_o = po2.tile([128, 512], F32, tag="po",
                                            name=f"po{h}_{n}")
                            ps_z = pz2.tile([1, 512], F32, tag="pz",
                                            name=f"pz{h}_{n}")
                            acc = npool.tile([128, 512], BF16, tag="zacc",
                                             bufs=2, name=f"zacc{h}_{n}")
                            nc.vector.memset(acc[:], 0.0)
                            # j processed descending so diagonal tiles
                            # (width-narrowed) come first and the last
                            # PSUM-group matmul (j=0) is full width.
                            js = list(range(jmax - 1, -1, -1))
                            pairs = [(js[2 * i], js[2 * i + 1])
                                     for i in range(jmax // 2)]

                            def left_of(j):
                                return (j - 4 * n) * 128 \
                                    if (j // 4) == n else 0

                            def emit_pv(pair, pexp):
                                for u, j in enumerate(pair):
                                    lf = left_of(j)
                                    psl = slice(u * 512 + lf, (u + 1) * 512)
                                    nc.tensor.matmul(
                                        ps_o[:, lf:512],
                                        v_sb[j][:, h * 128:(h + 1) * 128],
                                        pexp[:, psl],
                                        start=(j == jmax - 1), stop=(j == 0))

                            state = {}

                            def step(i):
                                pair = pairs[i]
                                ps_s = ps2.tile([128, 1024], F32, tag="ps",
                                                name=f"ps{h}_{n}_{pair[0]}")
                                for u, j in enumerate(pair):
                                    lf = left_of(j)
                                    diag = (j // 4) == n
                                    nc.tensor.matmul(
                                        ps_s[:, u * 512 + lf:(u + 1) * 512],
                                        kr[:, j * 128:(j + 1) * 128],
                                        qr[:, n * 512 + lf:(n + 1) * 512],
                                        start=True, stop=not diag)
                                    if diag:
                                        nc.tensor.matmul(
                                            ps_s[:, u * 512 + lf:
                                                 u * 512 + lf + 128],
                                            ident_sb[:], tri_sb[:],
                                            start=False, stop=True)
                                pexp = epool.tile(
                                    [128, 1024], BF16, tag="pexp", bufs=4,
                                    name=f"pexp{h}_{n}_{pair[0]}")
                                w0 = 512 - left_of(pair[0])
                                w1 = 512 - left_of(pair[1])
                                if w0 + w1 < 672:
                                    # two narrow calls beat one full call
                                    for u, j in enumerate(pair):
                                        lf = left_of(j)
                                        psl = slice(u * 512 + lf,
                                                    (u + 1) * 512)
                                        nc.scalar.activation(
                                            pexp[:, psl], ps_s[:, psl],
                                            mybir.ActivationFunctionType.Exp,
                                            scale=SCALE)
                                else:
                                    nc.scalar.activation(
                                        pexp[:], ps_s[:],
                                        mybir.ActivationFunctionType.Exp,
                                        scale=SCALE)
                                for u, j in enumerate(pair):
                                    lf = left_of(j)
                                    psl = slice(u * 512 + lf, (u + 1) * 512)
                                    nc.vector.tensor_add(
                                        acc[:, lf:512], acc[:, lf:512],
                                        pexp[:, psl])
                                prev = state.get("prev")
                                if prev is not None:
                                    emit_pv(*prev)
                                state["prev"] = (pair, pexp)

                            def tail():
                                emit_pv(*state["prev"])
                                nc.tensor.matmul(
                                    ps_z[:], onescol_sb[:], acc[:],
                                    start=True, stop=True)
                                qsl = slice(n * 512, (n + 1) * 512)
                                rz = npool.tile([1, 512], F32, tag="rz",
                                                bufs=2, name=f"rz{h}_{n}")
                                nc.vector.reciprocal(rz[:], ps_z[:])
                                # evict unnormalized P@V now so ps_o frees
                                # without waiting on the gpsimd broadcast
                                nc.vector.tensor_scalar_mul(
                                    attn_sb[h][:, qsl], ps_o[:], 1.0)
                                rzb = npool.tile([128, 512], F32, tag="rzb",
                                                 bufs=2, name=f"rzb{h}_{n}")
                                nc.gpsimd.partition_broadcast(rzb[:], rz[:])

                                def mul(h=h, qsl=qsl, rzb=rzb):
                                    nc.vector.tensor_mul(
                                        attn_sb[h][:, qsl],
                                        attn_sb[h][:, qsl], rzb[:])
                                pending_muls.append(mul)

                            return len(pairs), step, tail

                        pending_muls = []
                        for hp in range(HG // 2):
                            ha, hb = 2 * hp, 2 * hp + 1
                            for n in range(TC):
                                npairs, step_a, tail_a = make_chunk(ha, n)
                                _, step_b, tail_b = make_chunk(hb, n)
                                for i in range(npairs):
                                    step_a(i)
                                    step_b(i)
                                    if i == 0:
                                        # deferred normalize muls from the
                                        # previous chunk: far from both the
                                        # gpsimd broadcast and the evicts
                                        while pending_muls:
                                            pending_muls.pop(0)()
                                    # rope for the next head pair emitted
                                    # mid-stream so it doesn't block norm
                                    # ops in the DVE FIFO at head start
                                    if n == 1 and i == 1 and ha + 2 < HG:
                                        emit_rope(ha + 2)
                                    if n == 2 and i == 1 and hb + 2 < HG:
                                        emit_rope(hb + 2)
                                tail_a()
                                tail_b()
                        while pending_muls:
                            pending_muls.pop(0)()

                    # ---- Phase 3: out projection ----
                    with tc.tile_pool(name="yp", bufs=1) as ypool, \
                         tc.tile_pool(name="ps3", bufs=1,
                                      space="PSUM") as ps3:
                        for n in range(4):
                            if n == 0:
                                woc = woc0
                            else:
                                woc = w3pool.tile([128, HG * 512], BF16,
                                                  tag="woc", bufs=2,
                                                  name=f"woc{n}")
                                nc.sync.dma_start(
                                    out=woc[:],
                                    in_=wo[:, n * HG * 512:
                                          (n + 1) * HG * 512])
                            for m in range(TT):
                                ps_y = ps3.tile([128, 512], F32, tag="py",
                                                bufs=4, name=f"py{n}_{m}")
                                for h in range(HG):
                                    nc.tensor.matmul(
                                        ps_y[:],
                                        attn_sb[h][:, m * 128:(m + 1) * 128],
                                        woc[:, h * 512:(h + 1) * 512],
                                        start=(h == 0), stop=(h == HG - 1))
                                yt = ypool.tile([128, 512], F32, tag="yt",
                                                bufs=3, name=f"yt{n}_{m}")
                                nc.scalar.copy(yt[:], ps_y[:])
                                nc.sync.dma_start(
                                    out=y[m * 128:(m + 1) * 128,
                                          n * 512:(n + 1) * 512],
                                    in_=yt[:])

            if iters == 1:
                full_body(None)
            else:
                with tc.For_i(0, iters, 1) as iv:
                    full_body(iv)

    nc.compile()
    return nc


def make_host_inputs(x, Wqkv, bqkv, Wo):
    """Per-core input maps (host-side sharding + bf16 conversion)."""
    half = D // 2
    freq = np.arange(half, dtype=np.float64)
    theta = 1.0 / (10000.0 ** (2.0 * freq / D))
    pos = np.arange(T, dtype=np.float64)
    ang = pos[:, None] * theta[None, :]          # [T, half]
    sinT = np.sin(ang).T.astype(np.float32)      # [half, T]
    cosT = np.cos(ang).T.astype(np.float32)
    # sign folded into the sin table for the partition-swap RoPE form
    sin2 = np.concatenate([-sinT, sinT], axis=0).astype(BF_NP)  # [128, T]
    cos2 = np.concatenate([cosT, cosT], axis=0).astype(BF_NP)

    f = np.arange(128)[None, :]
    p = np.arange(128)[:, None]
    tri = np.where(f >= p, 0.0, MASKVAL).astype(BF_NP)
    ident = np.eye(128, dtype=np.float32).astype(BF_NP)
    onescol = np.ones((128, 1), dtype=np.float32).astype(BF_NP)
    ones128 = np.ones((1, 128), dtype=np.float32).astype(BF_NP)

    xT = [np.ascontiguousarray(x[b].T).astype(BF_NP) for b in range(B)]
    in_maps = []
    for core in range(N_CORES):
        b, g = core // 2, core % 2
        cs = slice(g * GC, (g + 1) * GC)
        Wq = Wqkv[:, :C][:, cs]
        Wk = Wqkv[:, C:2 * C][:, cs]
        Wv = Wqkv[:, 2 * C:][:, cs]
        # [C, 2*GC] -> [p, m, k, c] -> [128, 16*16*128]
        Wqk = np.concatenate([Wq, Wk], axis=1)
        wqk_r = np.ascontiguousarray(
            Wqk.reshape(KT, 128, 16, 128).transpose(1, 2, 0, 3)
            .reshape(128, 16 * KT * 128)).astype(BF_NP)
        bqk_r = np.concatenate(
            [bqkv[:C][cs], bqkv[C:2 * C][cs]]).reshape(2 * GC, 1)
        bqk_r = np.ascontiguousarray(bqk_r).astype(np.float32)
        # [C, GC] -> [p, nd, k, c] -> [128, 2*16*512]
        wv_r = np.ascontiguousarray(
            Wv.reshape(KT, 128, ND, 512).transpose(1, 2, 0, 3)
            .reshape(128, ND * KT * 512)).astype(BF_NP)
        bv_r = np.ascontiguousarray(
            bqkv[2 * C:][cs].reshape(1, GC)).astype(BF_NP)
        # [GC, C] -> [p, n, h, c] -> [128, 4*8*512]
        wo_r = np.ascontiguousarray(
            Wo[cs, :].reshape(HG, 128, 4, 512).transpose(1, 2, 0, 3)
            .reshape(128, 4 * HG * 512)).astype(BF_NP)
        in_maps.append({
            "xT": xT[b],
            "wqk": wqk_r,
            "bqk": bqk_r,
            "wv": wv_r,
            "bv": bv_r,
            "wo": wo_r,
            "sin2": sin2,
            "cos2": cos2,
            "tri": tri,
            "ident": ident,
            "onescol": onescol,
            "ones128": ones128,
        })
    return in_maps


_PROGRAM_CACHE = {}


def get_program(iters=1):
    if iters not in _PROGRAM_CACHE:
        _PROGRAM_CACHE[iters] = build_program(iters)
    return _PROGRAM_CACHE[iters]


def kernel(x, Wqkv, bqkv, Wo, bo):
    x = np.asarray(x, dtype=np.float32)
    Wqkv = np.asarray(Wqkv, dtype=np.float32)
    bqkv = np.asarray(bqkv, dtype=np.float32)
    Wo = np.asarray(Wo, dtype=np.float32)
    bo = np.asarray(bo, dtype=np.float32)

    nc = get_program(1)
    in_maps = make_host_inputs(x, Wqkv, bqkv, Wo)
    res = run_bass_kernel_spmd(nc, in_maps, list(range(N_CORES)))

    out = np.empty((B, T, C), dtype=np.float32)
    for b in range(B):
        out[b] = res.results[2 * b]["y"] + res.results[2 * b + 1]["y"] + bo
    return out
